# revision 1
# baseline (speedup 1.0000x reference)
"""GraphSAGE 2-layer fraud detector on 8 trn2 NeuronCores.

Strategy (dst-partitioned, matmul scatter):
  - Nodes padded to 50176 = 8 cores x 49 blocks x 128. Core c owns dst rows
    [c*6272, (c+1)*6272). Edges sorted by dst on host; each core gets the
    edges targeting its rows, grouped per 128-node dst block, chunked into
    128-edge chunks.
  - Layer 1 per chunk: indirect-DMA gather x[src] rows (512B each), build
    one-hot P[e,d] = (ldst[e]==d) on DVE, PSUM-accumulate P.T @ msg ->
    agg[dst,feat]. Mean via per-partition recip scale, then
    h = relu(agg@W1l.T + x@W1r.T + b1) computed feature-major (hT) via a PE
    transpose of agg.
  - z = h@W2l.T and o = h@W2r.T + b2 computed per block (mean-aggregation
    commutes with the linear map, so layer 2 aggregates the 2-wide z instead
    of the 256-wide h). z is AllGathered across cores (50KB/core); layer 2
    reuses the same chunk tables to gather z[src] rows and the same one-hot
    scatter into agg2[dst,2]. out = recip*agg2 + o.
"""

import time

import numpy as np

import concourse.bass as bass
import concourse.mybir as mybir
import concourse.tile as tile
from concourse import bacc
from concourse.bass_utils import run_bass_kernel_spmd

N = 50000
E = 800000
IN_C = 128
HID = 256
OUT_C = 2
NCORES = 8
P = 128
NB = 49                 # dst blocks per core
ROWS = NB * P           # 6272 rows per core
NP = NCORES * ROWS      # 50176 padded nodes
ZROWS = NCORES * P      # 1024 rows of the allgathered z tensor [1024, 2*NB]

f32 = mybir.dt.float32
i32 = mybir.dt.int32

DEBUG_TAPS = False


def _host_prep(x, edge_index, W1l, b1, W1r, W2l, b2, W2r):
    src = edge_index[0].astype(np.int64)
    dst = edge_index[1].astype(np.int64)
    cnt = np.bincount(dst, minlength=NP)
    recip = (1.0 / np.maximum(cnt, 1)).astype(np.float32)

    order = np.argsort(dst, kind="stable")
    s_src = src[order]
    s_dst = dst[order]

    block_starts = np.searchsorted(s_dst, np.arange(0, NP + P, P))
    cnt_blk = (block_starts[1:] - block_starts[:-1]).reshape(NCORES, NB)
    nb = np.maximum(1, -(-cnt_blk // P)).max(axis=0)  # [NB] chunks per block pos
    C1 = int(nb.sum())

    src_arr = np.full((NCORES, P, C1), N, dtype=np.int32)
    ldst_arr = np.full((NCORES, P, C1), 255, dtype=np.float32)
    col = 0
    for b in range(NB):
        w = int(nb[b])
        for c in range(NCORES):
            bb = c * NB + b
            s, e = int(block_starts[bb]), int(block_starts[bb + 1])
            k = e - s
            ts = np.full(w * P, N, np.int32)
            tl = np.full(w * P, 255, np.float32)
            ts[:k] = s_src[s:e]
            tl[:k] = s_dst[s:e] - bb * P
            src_arr[c, :, col:col + w] = ts.reshape(w, P).T
            ldst_arr[c, :, col:col + w] = tl.reshape(w, P).T
        col += w

    # layer-2 gathers the same edges from z_full, whose row layout is
    # [core, p, b]: node n lives at flat row (n//6272)*6272 + (n%128)*49
    # + ((n%6272)//128). Pad entries point at node N; their one-hot column
    # is zero so the gathered value never contributes.
    sa = src_arr.astype(np.int64)
    qsrc_arr = ((sa // ROWS) * ROWS + (sa % P) * NB
                + (sa % ROWS) // P).astype(np.int32)

    x_pad = np.zeros((NP + 1, IN_C), np.float32)
    x_pad[:N] = x
    W1lT = np.ascontiguousarray(W1l.T.astype(np.float32))   # [128, 256]
    W1rT = np.ascontiguousarray(W1r.T.astype(np.float32))
    Wzo = np.zeros((P, 8), np.float32)
    for j in range(2):
        Wzo[:, 4 * j:4 * j + 2] = W2l.T[j * P:(j + 1) * P, :]
        Wzo[:, 4 * j + 2:4 * j + 4] = W2r.T[j * P:(j + 1) * P, :]
    b1p = np.ascontiguousarray(np.asarray(b1).reshape(2, P).T.astype(np.float32))
    b2b = np.tile(np.asarray(b2).reshape(1, 2), (P, 1)).astype(np.float32)
    recip_c = recip.reshape(NCORES, NB, P).transpose(0, 2, 1).copy()  # [c,P,NB]
    iota = np.tile(np.arange(P, dtype=np.float32)[None, :], (P, 1))
    ident = np.eye(P, dtype=np.float32)

    in_maps = []
    for c in range(NCORES):
        xT_own = np.ascontiguousarray(
            x_pad[c * ROWS:(c + 1) * ROWS, :].T
        )  # [128, 6272]
        in_maps.append({
            "x_pad": x_pad,
            "src": np.ascontiguousarray(src_arr[c]),
            "ldst": np.ascontiguousarray(ldst_arr[c]),
            "qsrc": np.ascontiguousarray(qsrc_arr[c]),
            "xT_own": xT_own,
            "W1lT": W1lT,
            "W1rT": W1rT,
            "Wzo": Wzo,
            "b1p": b1p,
            "b2b": b2b,
            "recip": np.ascontiguousarray(recip_c[c]),
            "iota": iota,
            "ident": ident,
        })
    return in_maps, [int(v) for v in nb]


def _build(nb):
    C1 = sum(nb)
    nc = bacc.Bacc(None, target_bir_lowering=False, debug=False)

    x_pad_d = nc.dram_tensor("x_pad", [NP + 1, IN_C], f32, kind="ExternalInput")
    src_d = nc.dram_tensor("src", [P, C1], i32, kind="ExternalInput")
    ldst_d = nc.dram_tensor("ldst", [P, C1], f32, kind="ExternalInput")
    qsrc_d = nc.dram_tensor("qsrc", [P, C1], i32, kind="ExternalInput")
    xT_d = nc.dram_tensor("xT_own", [P, ROWS], f32, kind="ExternalInput")
    W1lT_d = nc.dram_tensor("W1lT", [P, HID], f32, kind="ExternalInput")
    W1rT_d = nc.dram_tensor("W1rT", [P, HID], f32, kind="ExternalInput")
    Wzo_d = nc.dram_tensor("Wzo", [P, 8], f32, kind="ExternalInput")
    b1p_d = nc.dram_tensor("b1p", [P, 2], f32, kind="ExternalInput")
    b2b_d = nc.dram_tensor("b2b", [P, 2], f32, kind="ExternalInput")
    recip_d = nc.dram_tensor("recip", [P, NB], f32, kind="ExternalInput")
    iota_d = nc.dram_tensor("iota", [P, P], f32, kind="ExternalInput")
    ident_d = nc.dram_tensor("ident", [P, P], f32, kind="ExternalInput")
    out_d = nc.dram_tensor("out", [P, 2 * NB], f32, kind="ExternalOutput")
    if DEBUG_TAPS:
        dbg_msg_d = nc.dram_tensor("dbg_msg", [P, nb[0] * P], f32,
                                   kind="ExternalOutput")
        dbg_aggm_d = nc.dram_tensor("dbg_aggm", [P, P], f32,
                                    kind="ExternalOutput")
        dbg_h0_d = nc.dram_tensor("dbg_h0", [P, P], f32, kind="ExternalOutput")
        dbg_z_d = nc.dram_tensor("dbg_z", [P, 2 * NB], f32,
                                 kind="ExternalOutput")

    with tile.TileContext(nc) as tc:
        with (
            tc.tile_pool(name="big", bufs=1) as big,
            tc.tile_pool(name="lp", bufs=4) as lp,
            tc.tile_pool(name="pp", bufs=2, space="PSUM") as pp,
            tc.tile_pool(name="dram", bufs=1, space="DRAM") as dp,
        ):
            def load(d, shape, dt, tag):
                t = big.tile(shape, dt, tag=tag)
                nc.sync.dma_start(out=t[:], in_=d[:, :])
                return t

            src_sb = load(src_d, [P, C1], i32, "src")
            ldst_sb = load(ldst_d, [P, C1], f32, "ldst")
            qsrc_sb = load(qsrc_d, [P, C1], i32, "qsrc")
            xT_sb = load(xT_d, [P, ROWS], f32, "xT")
            W1lT_sb = load(W1lT_d, [P, HID], f32, "w1l")
            W1rT_sb = load(W1rT_d, [P, HID], f32, "w1r")
            Wzo_sb = load(Wzo_d, [P, 8], f32, "wzo")
            b1_sb = load(b1p_d, [P, 2], f32, "b1")
            b2_sb = load(b2b_d, [P, 2], f32, "b2")
            recip_sb = load(recip_d, [P, NB], f32, "recip")
            iota_sb = load(iota_d, [P, P], f32, "iota")
            ident_sb = load(ident_d, [P, P], f32, "ident")

            hT = [
                big.tile([P, ROWS], f32, tag=f"hT{j}", name=f"hT{j}")
                for j in range(2)
            ]
            z_sb = big.tile([P, 2 * NB], f32, tag="z")
            o_sb = big.tile([P, 2 * NB], f32, tag="o")
            out_sb = big.tile([P, 2 * NB], f32, tag="outs")

            col = 0
            for b in range(NB):
                w = nb[b]
                pagg = pp.tile([P, P], f32, tag="agg")
                for k in range(w):
                    msg = lp.tile([P, P], f32, tag="msg")
                    nc.gpsimd.indirect_dma_start(
                        out=msg[:],
                        out_offset=None,
                        in_=x_pad_d[:, :],
                        in_offset=bass.IndirectOffsetOnAxis(
                            ap=src_sb[:, col + k:col + k + 1], axis=0
                        ),
                    )
                    if DEBUG_TAPS and b == 0:
                        nc.sync.dma_start(
                            out=dbg_msg_d[:, k * P:(k + 1) * P], in_=msg[:]
                        )
                    Pt = lp.tile([P, P], f32, tag="P")
                    nc.vector.tensor_scalar(
                        out=Pt[:], in0=iota_sb[:],
                        scalar1=ldst_sb[:, col + k:col + k + 1], scalar2=None,
                        op0=mybir.AluOpType.is_equal,
                    )
                    nc.tensor.matmul(
                        out=pagg[:], lhsT=Pt[:], rhs=msg[:],
                        start=(k == 0), stop=(k == w - 1),
                    )
                aggm = lp.tile([P, P], f32, tag="aggm")
                nc.vector.tensor_scalar(
                    out=aggm[:], in0=pagg[:], scalar1=recip_sb[:, b:b + 1],
                    scalar2=None, op0=mybir.AluOpType.mult,
                )
                if DEBUG_TAPS and b == 0:
                    nc.sync.dma_start(out=dbg_aggm_d[:, :], in_=aggm[:])
                ptr = pp.tile([P, P], f32, tag="tr")
                nc.tensor.transpose(out=ptr[:], in_=aggm[:], identity=ident_sb[:])
                aggmT = lp.tile([P, P], f32, tag="aggmT")
                nc.vector.tensor_copy(out=aggmT[:], in_=ptr[:])
                for j in range(2):
                    ph = pp.tile([P, P], f32, tag="h")
                    nc.tensor.matmul(
                        out=ph[:], lhsT=W1lT_sb[:, j * P:(j + 1) * P],
                        rhs=aggmT[:], start=True, stop=False,
                    )
                    nc.tensor.matmul(
                        out=ph[:], lhsT=W1rT_sb[:, j * P:(j + 1) * P],
                        rhs=xT_sb[:, b * P:(b + 1) * P], start=False, stop=True,
                    )
                    nc.scalar.activation(
                        out=hT[j][:, b * P:(b + 1) * P], in_=ph[:],
                        func=mybir.ActivationFunctionType.Relu,
                        bias=b1_sb[:, j:j + 1],
                    )
                if DEBUG_TAPS and b == 0:
                    nc.sync.dma_start(out=dbg_h0_d[:, :], in_=hT[0][:, 0:P])
                pzo = pp.tile([P, 4], f32, tag="zo")
                for j in range(2):
                    nc.tensor.matmul(
                        out=pzo[:], lhsT=hT[j][:, b * P:(b + 1) * P],
                        rhs=Wzo_sb[:, 4 * j:4 * j + 4],
                        start=(j == 0), stop=(j == 1),
                    )
                nc.vector.tensor_copy(out=z_sb[:, 2 * b:2 * b + 2], in_=pzo[:, 0:2])
                nc.vector.tensor_tensor(
                    out=o_sb[:, 2 * b:2 * b + 2], in0=pzo[:, 2:4], in1=b2_sb[:],
                    op=mybir.AluOpType.add,
                )
                col += w

            if DEBUG_TAPS:
                nc.sync.dma_start(out=dbg_z_d[:, :], in_=z_sb[:])

            # z -> DRAM, allgather
            z_own = dp.tile([P, 2 * NB], f32, tag="zown")
            nc.sync.dma_start(out=z_own[:], in_=z_sb[:])
            z_full = dp.tile([ZROWS, 2 * NB], f32, tag="zfull")
            nc.gpsimd.collective_compute(
                "AllGather",
                mybir.AluOpType.bypass,
                replica_groups=[list(range(NCORES))],
                ins=[z_own[:, :]],
                outs=[z_full[:, :]],
            )
            z_rows = z_full[:, :].rearrange("a (r f) -> (a r) f", f=2)

            col = 0
            for b in range(NB):
                w = nb[b]
                pa2 = pp.tile([P, 2], f32, tag="agg", name="pa2")
                for k in range(w):
                    zg = lp.tile([P, 2], f32, tag="zg")
                    nc.gpsimd.indirect_dma_start(
                        out=zg[:],
                        out_offset=None,
                        in_=z_rows,
                        in_offset=bass.IndirectOffsetOnAxis(
                            ap=qsrc_sb[:, col + k:col + k + 1], axis=0
                        ),
                    )
                    P2 = lp.tile([P, P], f32, tag="P2")
                    nc.vector.tensor_scalar(
                        out=P2[:], in0=iota_sb[:],
                        scalar1=ldst_sb[:, col + k:col + k + 1], scalar2=None,
                        op0=mybir.AluOpType.is_equal,
                    )
                    nc.tensor.matmul(
                        out=pa2[:], lhsT=P2[:], rhs=zg[:],
                        start=(k == 0), stop=(k == w - 1),
                    )
                red2 = lp.tile([P, 2], f32, tag="red2")
                nc.vector.tensor_scalar(
                    out=red2[:], in0=pa2[:], scalar1=recip_sb[:, b:b + 1],
                    scalar2=None, op0=mybir.AluOpType.mult,
                )
                nc.vector.tensor_tensor(
                    out=out_sb[:, 2 * b:2 * b + 2], in0=red2[:],
                    in1=o_sb[:, 2 * b:2 * b + 2], op=mybir.AluOpType.add,
                )
                col += w

            nc.sync.dma_start(out=out_d[:, :], in_=out_sb[:])
    nc.compile()
    return nc


def _run(inputs, repeat=1):
    in_maps, nb = _host_prep(**inputs)
    nc = _build(nb)
    best = None
    for _ in range(repeat):
        t0 = time.perf_counter()
        res = run_bass_kernel_spmd(
            nc, [dict(m) for m in in_maps], core_ids=list(range(NCORES))
        )
        dt = time.perf_counter() - t0
        best = dt if best is None else min(best, dt)
    outs = []
    for c in range(NCORES):
        a = res.results[c]["out"]  # [128, 98]
        outs.append(a.reshape(P, NB, 2).transpose(1, 0, 2).reshape(ROWS, 2))
    full = np.concatenate(outs, axis=0)[:N]
    return full.astype(np.float32), best


def kernel(**inputs):
    out, _ = _run(inputs, repeat=1)
    return out



# revision 4
# speedup vs baseline: 7.2219x; 7.2219x over previous
"""GraphSAGE 2-layer fraud detector on 8 trn2 NeuronCores.

Strategy (dst-partitioned, matmul scatter, minimal host->device traffic):
  - The axon tunnel moves ~45MB/s, so wall time is dominated by input bytes
    shipped per spmd call. Each core receives ONLY its x shard (fp16,
    1.6MB) plus compressed edge tables; x is AllGathered across cores
    on-device into DRAM, and everything else (iota, identity, x^T blocks)
    is derived on-device.
  - Nodes padded to 50176 = 8 cores x 49 blocks x 128. Core c owns nodes
    [c*6272, (c+1)*6272). Within a core, dst block b holds the 128 nodes
    with local index p*49 + b (p = row in block). This layout makes the
    SBUF z tile [128, 2*49] a plain contiguous view of z rows in node
    order, so the layer-2 gather reuses the SAME src table as layer 1.
  - Layer 1 per chunk of 128 edges (sorted by dst): indirect-DMA gather
    x_full[src] rows (256B fp16), build one-hot P[e,d] = (ldst[e]==d) on
    DVE, PSUM-accumulate P.T @ msg -> agg[dst,feat]. Mean via per-partition
    recip scale, then hT = relu(W1l @ aggT + W1r @ xbT + b1) per block via
    PE transposes of agg and the x block.
  - z = h@W2l.T and o = h@W2r.T + b2 per block (mean-aggregation commutes
    with the linear map, so layer 2 aggregates the 2-wide z instead of the
    256-wide h). z is AllGathered (node-ordered [50176, 2]); layer 2
    gathers z[src] with the same tables and scatters into agg2, then
    out = recip*agg2 + o.
"""

import time

import numpy as np

import concourse.bass as bass
import concourse.mybir as mybir
import concourse.tile as tile
from concourse import bacc
from concourse.bass_utils import run_bass_kernel_spmd

N = 50000
E = 800000
IN_C = 128
HID = 256
OUT_C = 2
NCORES = 8
P = 128
NB = 49                 # dst blocks per core
ROWS = NB * P           # 6272 rows per core
NP = NCORES * ROWS      # 50176 padded nodes

f32 = mybir.dt.float32
f16 = mybir.dt.float16
i32 = mybir.dt.int32
u16 = mybir.dt.uint16
u8 = mybir.dt.uint8


def _host_prep(x, edge_index, W1l, b1, W1r, W2l, b2, W2r):
    src = edge_index[0].astype(np.int64)
    dst = edge_index[1].astype(np.int64)
    cnt = np.bincount(dst, minlength=NP)
    recip = (1.0 / np.maximum(cnt, 1)).astype(np.float32)

    # dst sort key in block-layout space: node (core c, local r) sits in
    # block b = r % 49 at row p = r // 49 -> key = c*6272 + b*128 + p.
    c_ = dst // ROWS
    r_ = dst % ROWS
    key = c_ * ROWS + (r_ % NB) * P + (r_ // NB)
    order = np.argsort(key, kind="stable")
    s_src = src[order]
    s_key = key[order]

    block_starts = np.searchsorted(s_key, np.arange(0, NP + P, P))
    cnt_blk = (block_starts[1:] - block_starts[:-1]).reshape(NCORES, NB)
    nb = np.maximum(1, -(-cnt_blk // P)).max(axis=0)  # [NB] chunks per block pos
    C1 = int(nb.sum())

    src_arr = np.full((NCORES, P, C1), N, dtype=np.uint16)
    ldst_arr = np.full((NCORES, P, C1), 255, dtype=np.uint8)
    col = 0
    for b in range(NB):
        w = int(nb[b])
        for c in range(NCORES):
            bb = c * NB + b
            s, e = int(block_starts[bb]), int(block_starts[bb + 1])
            k = e - s
            ts = np.full(w * P, N, np.uint16)
            tl = np.full(w * P, 255, np.uint8)
            ts[:k] = s_src[s:e]
            tl[:k] = s_key[s:e] % P
            src_arr[c, :, col:col + w] = ts.reshape(w, P).T
            ldst_arr[c, :, col:col + w] = tl.reshape(w, P).T
        col += w

    x_pad = np.zeros((NP, IN_C), np.float16)
    x_pad[:N] = x.astype(np.float16)
    W1lT = np.ascontiguousarray(W1l.T.astype(np.float16))   # [128, 256]
    W1rT = np.ascontiguousarray(W1r.T.astype(np.float16))
    Wzo = np.zeros((P, 8), np.float16)
    for j in range(2):
        Wzo[:, 4 * j:4 * j + 2] = W2l.T[j * P:(j + 1) * P, :].astype(np.float16)
        Wzo[:, 4 * j + 2:4 * j + 4] = W2r.T[j * P:(j + 1) * P, :].astype(np.float16)
    b1p = np.ascontiguousarray(np.asarray(b1).reshape(2, P).T.astype(np.float32))
    b2b = np.tile(np.asarray(b2).reshape(1, 2), (P, 1)).astype(np.float32)
    # recip for node local r = p*49 + b at [p, b]
    recip_c = recip.reshape(NCORES, P, NB).copy()

    in_maps = []
    for c in range(NCORES):
        in_maps.append({
            "x_sh": np.ascontiguousarray(x_pad[c * ROWS:(c + 1) * ROWS, :]),
            "srcu": np.ascontiguousarray(src_arr[c]),
            "ldstu": np.ascontiguousarray(ldst_arr[c]),
            "W1lT": W1lT,
            "W1rT": W1rT,
            "Wzo": Wzo,
            "b1p": b1p,
            "b2b": b2b,
            "recip": np.ascontiguousarray(recip_c[c]),
        })
    return in_maps, [int(v) for v in nb]


def _build(nb):
    C1 = sum(nb)
    nc = bacc.Bacc(None, target_bir_lowering=False, debug=False)

    x_sh_d = nc.dram_tensor("x_sh", [ROWS, IN_C], f16, kind="ExternalInput")
    srcu_d = nc.dram_tensor("srcu", [P, C1], u16, kind="ExternalInput")
    ldstu_d = nc.dram_tensor("ldstu", [P, C1], u8, kind="ExternalInput")
    W1lT_d = nc.dram_tensor("W1lT", [P, HID], f16, kind="ExternalInput")
    W1rT_d = nc.dram_tensor("W1rT", [P, HID], f16, kind="ExternalInput")
    Wzo_d = nc.dram_tensor("Wzo", [P, 8], f16, kind="ExternalInput")
    b1p_d = nc.dram_tensor("b1p", [P, 2], f32, kind="ExternalInput")
    b2b_d = nc.dram_tensor("b2b", [P, 2], f32, kind="ExternalInput")
    recip_d = nc.dram_tensor("recip", [P, NB], f32, kind="ExternalInput")
    out_d = nc.dram_tensor("out", [P, 2 * NB], f32, kind="ExternalOutput")

    with tile.TileContext(nc) as tc:
        with (
            tc.tile_pool(name="big", bufs=1) as big,
            tc.tile_pool(name="lp", bufs=4) as lp,
            tc.tile_pool(name="pp", bufs=2, space="PSUM") as pp,
            tc.tile_pool(name="dram", bufs=1, space="DRAM") as dp,
        ):
            def load(d, shape, dt, tag):
                t = big.tile(shape, dt, tag=tag, name=tag)
                nc.sync.dma_start(out=t[:], in_=d[:, :])
                return t

            srcu_sb = load(srcu_d, [P, C1], u16, "srcu")
            ldstu_sb = load(ldstu_d, [P, C1], u8, "ldstu")
            W1lT_sb = load(W1lT_d, [P, HID], f16, "w1l")
            W1rT_sb = load(W1rT_d, [P, HID], f16, "w1r")
            Wzo_sb = load(Wzo_d, [P, 8], f16, "wzo")
            b1_sb = load(b1p_d, [P, 2], f32, "b1")
            b2_sb = load(b2b_d, [P, 2], f32, "b2")
            recip_sb = load(recip_d, [P, NB], f32, "recip")

            # widen the compressed tables
            src_sb = big.tile([P, C1], i32, tag="src", name="src_sb")
            nc.vector.tensor_copy(out=src_sb[:], in_=srcu_sb[:])
            ldst_sb = big.tile([P, C1], f32, tag="ldst", name="ldst_sb")
            nc.vector.tensor_copy(out=ldst_sb[:], in_=ldstu_sb[:])

            # iota / identity built on device
            ioti = big.tile([P, P], i32, tag="ioti", name="ioti")
            nc.gpsimd.iota(out=ioti[:], pattern=[[1, P]], base=0,
                           channel_multiplier=0)
            iotp = big.tile([P, P], i32, tag="iotp", name="iotp")
            nc.gpsimd.iota(out=iotp[:], pattern=[[0, P]], base=0,
                           channel_multiplier=1)
            iota_sb = big.tile([P, P], f32, tag="iota", name="iota_sb")
            nc.vector.tensor_copy(out=iota_sb[:], in_=ioti[:])
            identh = big.tile([P, P], f16, tag="identh", name="identh")
            nc.vector.tensor_tensor(
                out=identh[:], in0=ioti[:], in1=iotp[:],
                op=mybir.AluOpType.is_equal,
            )

            # x shard -> internal DRAM -> AllGather to full x
            x_int = dp.tile([ROWS, IN_C], f16, tag="xint", name="x_int")
            nc.sync.dma_start(out=x_int[:, :], in_=x_sh_d[:, :])
            x_full = dp.tile([NP, IN_C], f16, tag="xfull", name="x_full",
                             addr_space="Shared")
            nc.gpsimd.collective_compute(
                "AllGather",
                mybir.AluOpType.bypass,
                replica_groups=[list(range(NCORES))],
                ins=[x_int[:, :]],
                outs=[x_full[:, :]],
            )
            xb_src = x_int[:, :].rearrange("(p b) c -> p b c", b=NB)

            z_sb = big.tile([P, 2 * NB], f32, tag="z", name="z_sb")
            o_sb = big.tile([P, 2 * NB], f32, tag="o", name="o_sb")
            out_sb = big.tile([P, 2 * NB], f32, tag="outs", name="out_sb")

            col = 0
            for b in range(NB):
                w = nb[b]
                pagg = pp.tile([P, P], f32, tag="agg", name="pagg")
                for k in range(w):
                    msg = lp.tile([P, IN_C], f16, tag="msg", name="msg")
                    nc.gpsimd.indirect_dma_start(
                        out=msg[:],
                        out_offset=None,
                        in_=x_full[:, :],
                        in_offset=bass.IndirectOffsetOnAxis(
                            ap=src_sb[:, col + k:col + k + 1], axis=0
                        ),
                    )
                    Pt = lp.tile([P, P], f16, tag="P", name="Pt")
                    nc.vector.tensor_scalar(
                        out=Pt[:], in0=iota_sb[:],
                        scalar1=ldst_sb[:, col + k:col + k + 1], scalar2=None,
                        op0=mybir.AluOpType.is_equal,
                    )
                    nc.tensor.matmul(
                        out=pagg[:], lhsT=Pt[:], rhs=msg[:],
                        start=(k == 0), stop=(k == w - 1),
                    )
                aggm = lp.tile([P, P], f16, tag="aggm", name="aggm")
                nc.vector.tensor_scalar(
                    out=aggm[:], in0=pagg[:], scalar1=recip_sb[:, b:b + 1],
                    scalar2=None, op0=mybir.AluOpType.mult,
                )
                ptr = pp.tile([P, P], f16, tag="tr", name="ptr", bufs=3)
                nc.tensor.transpose(out=ptr[:], in_=aggm[:], identity=identh[:])
                aggmT = lp.tile([P, P], f16, tag="aggmT", name="aggmT")
                nc.vector.tensor_copy(out=aggmT[:], in_=ptr[:])

                xb = lp.tile([P, IN_C], f16, tag="xb", name="xb")
                nc.sync.dma_start(out=xb[:], in_=xb_src[:, b, :])
                ptr2 = pp.tile([P, P], f16, tag="tr", name="ptr2", bufs=3)
                nc.tensor.transpose(out=ptr2[:], in_=xb[:], identity=identh[:])
                xbT = lp.tile([P, P], f16, tag="xbT", name="xbT")
                nc.vector.tensor_copy(out=xbT[:], in_=ptr2[:])

                hbT = []
                for j in range(2):
                    ph = pp.tile([P, P], f32, tag="tr", name="ph", bufs=3)
                    nc.tensor.matmul(
                        out=ph[:], lhsT=W1lT_sb[:, j * P:(j + 1) * P],
                        rhs=aggmT[:], start=True, stop=False,
                    )
                    nc.tensor.matmul(
                        out=ph[:], lhsT=W1rT_sb[:, j * P:(j + 1) * P],
                        rhs=xbT[:], start=False, stop=True,
                    )
                    ht = lp.tile([P, P], f16, tag=f"hbT{j}", name=f"ht{j}")
                    nc.scalar.activation(
                        out=ht[:], in_=ph[:],
                        func=mybir.ActivationFunctionType.Relu,
                        bias=b1_sb[:, j:j + 1],
                    )
                    hbT.append(ht)
                pzo = pp.tile([P, 4], f32, tag="zo", name="pzo", bufs=1)
                for j in range(2):
                    nc.tensor.matmul(
                        out=pzo[:], lhsT=hbT[j][:],
                        rhs=Wzo_sb[:, 4 * j:4 * j + 4],
                        start=(j == 0), stop=(j == 1),
                    )
                nc.vector.tensor_copy(out=z_sb[:, 2 * b:2 * b + 2], in_=pzo[:, 0:2])
                nc.vector.tensor_tensor(
                    out=o_sb[:, 2 * b:2 * b + 2], in0=pzo[:, 2:4], in1=b2_sb[:],
                    op=mybir.AluOpType.add,
                )
                col += w

            # z -> DRAM in node order (contiguous view), allgather
            z_own = dp.tile([ROWS, 2], f32, tag="zown", name="z_own")
            nc.sync.dma_start(
                out=z_own[:, :].rearrange("(p b) f -> p (b f)", b=NB),
                in_=z_sb[:],
            )
            z_full = dp.tile([NP, 2], f32, tag="zfull", name="z_full",
                             addr_space="Shared")
            nc.gpsimd.collective_compute(
                "AllGather",
                mybir.AluOpType.bypass,
                replica_groups=[list(range(NCORES))],
                ins=[z_own[:, :]],
                outs=[z_full[:, :]],
            )

            col = 0
            for b in range(NB):
                w = nb[b]
                pa2 = pp.tile([P, 2], f32, tag="agg2", name="pa2")
                for k in range(w):
                    zg = lp.tile([P, 2], f32, tag="zg", name="zg")
                    nc.gpsimd.indirect_dma_start(
                        out=zg[:],
                        out_offset=None,
                        in_=z_full[:, :],
                        in_offset=bass.IndirectOffsetOnAxis(
                            ap=src_sb[:, col + k:col + k + 1], axis=0
                        ),
                    )
                    P2 = lp.tile([P, P], f32, tag="P2", name="P2")
                    nc.vector.tensor_scalar(
                        out=P2[:], in0=iota_sb[:],
                        scalar1=ldst_sb[:, col + k:col + k + 1], scalar2=None,
                        op0=mybir.AluOpType.is_equal,
                    )
                    nc.tensor.matmul(
                        out=pa2[:], lhsT=P2[:], rhs=zg[:],
                        start=(k == 0), stop=(k == w - 1),
                    )
                red2 = lp.tile([P, 2], f32, tag="red2", name="red2")
                nc.vector.tensor_scalar(
                    out=red2[:], in0=pa2[:], scalar1=recip_sb[:, b:b + 1],
                    scalar2=None, op0=mybir.AluOpType.mult,
                )
                nc.vector.tensor_tensor(
                    out=out_sb[:, 2 * b:2 * b + 2], in0=red2[:],
                    in1=o_sb[:, 2 * b:2 * b + 2], op=mybir.AluOpType.add,
                )
                col += w

            nc.sync.dma_start(out=out_d[:, :], in_=out_sb[:])
    nc.compile()
    return nc


def _run(inputs, repeat=1):
    in_maps, nb = _host_prep(**inputs)
    nc = _build(nb)
    best = None
    for _ in range(repeat):
        t0 = time.perf_counter()
        res = run_bass_kernel_spmd(
            nc, [dict(m) for m in in_maps], core_ids=list(range(NCORES))
        )
        dt = time.perf_counter() - t0
        print(f"  spmd run: {dt:.3f}s", flush=True)
        best = dt if best is None else min(best, dt)
    outs = []
    for c in range(NCORES):
        a = res.results[c]["out"]  # [128, 98]; row p, col 2b+f = node p*49+b
        outs.append(a.reshape(ROWS, 2))
    full = np.concatenate(outs, axis=0)[:N]
    return full.astype(np.float32), best


def kernel(**inputs):
    out, _ = _run(inputs, repeat=1)
    return out


# revision 7
# speedup vs baseline: 20.7041x; 2.8668x over previous
"""GraphSAGE 2-layer fraud detector on 8 trn2 NeuronCores.

Strategy (dst-partitioned, matmul scatter, minimal host->device traffic):
  - The axon tunnel moves ~45MB/s, so wall time is dominated by (a) input
    bytes shipped per spmd call and (b) per-call re-lowering of the kernel
    BIR (proportional to instruction count). Each core receives ONLY its
    x shard (fp16, 1.6MB) plus compressed edge tables; x is AllGathered
    across cores on-device into DRAM, and everything else (iota, identity,
    x^T blocks) is derived on-device.
  - Nodes padded to 50176 = 8 cores x 49 blocks x 128. Core c owns nodes
    [c*6272, (c+1)*6272). Within a core, dst block b holds the 128 nodes
    with local index p*49 + b (p = row in block), which makes the z tile a
    plain contiguous view of z rows in node order.
  - All per-edge work is driven by gpsimd.dma_gather: one instruction
    gathers a whole block's worth of 256B rows from an HBM table into
    SBUF [128, chunks, 128]. Indices are int16, so x_full is addressed as
    two half-tables (rows [0,25088) and [25088,50176)) and each block's
    edges are partitioned by source half (order within a block is free).
  - Layer 1, per block (hardware For_i loop, ~55 instrs total): gather
    msg rows, build one-hot P[e,d]=(ldst[e]==d) per 128-edge chunk on DVE,
    PSUM-accumulate P.T @ msg = agg, scale by 1/deg, then
    hT = relu(W1l @ aggT + W1r @ xbT + b1) via PE transposes.
  - z = h@W2l.T, o = h@W2r.T + b2 (aggregation commutes with the linear
    map, so layer 2 aggregates 2-wide z, not 256-wide h). z is written
    into a 256B-padded node-ordered table, AllGathered, and layer 2 reuses
    the SAME index tables to gather z and scatter into agg2;
    out = recip*agg2 + o.
"""

import time

import numpy as np

import concourse.bass as bass
import concourse.mybir as mybir
import concourse.tile as tile
from concourse import bacc
from concourse.bass import ds, ts
from concourse.bass_utils import run_bass_kernel_spmd

N = 50000
E = 800000
IN_C = 128
HID = 256
OUT_C = 2
NCORES = 8
P = 128
NB = 49                 # dst blocks per core
ROWS = NB * P           # 6272 rows per core
NP = NCORES * ROWS      # 50176 padded nodes
HALF = NP // 2          # 25088 rows per half-table (int16-addressable)

f32 = mybir.dt.float32
f16 = mybir.dt.float16
i32 = mybir.dt.int32
i16 = mybir.dt.int16
u8 = mybir.dt.uint8


def _wrap16(flat):
    """dma_gather index layout: flat j -> [partition j%16, col j//16]."""
    return np.ascontiguousarray(flat.reshape(-1, 16).T)


def _host_prep(x, edge_index, W1l, b1, W1r, W2l, b2, W2r):
    src = edge_index[0].astype(np.int64)
    dst = edge_index[1].astype(np.int64)
    cnt = np.bincount(dst, minlength=NP)
    recip = (1.0 / np.maximum(cnt, 1)).astype(np.float32)

    # dst sort key in block-layout space: node (core c, local r) sits in
    # block b = r % 49 at row p = r // 49 -> key = c*6272 + b*128 + p.
    c_ = dst // ROWS
    r_ = dst % ROWS
    key = c_ * ROWS + (r_ % NB) * P + (r_ // NB)
    order = np.argsort(key, kind="stable")
    s_src = src[order]
    s_key = key[order]

    block_starts = np.searchsorted(s_key, np.arange(0, NP + P, P))

    # per (core, block): split edges by source half, count chunks
    W0 = 1
    W1 = 1
    parts = {}
    for bb in range(NCORES * NB):
        s, e = int(block_starts[bb]), int(block_starts[bb + 1])
        bs = s_src[s:e]
        bl = (s_key[s:e] % P).astype(np.uint8)
        m0 = bs < HALF
        p0s, p0l = bs[m0], bl[m0]
        p1s, p1l = bs[~m0] - HALF, bl[~m0]
        parts[bb] = (p0s, p0l, p1s, p1l)
        W0 = max(W0, -(-len(p0s) // P))
        W1 = max(W1, -(-len(p1s) // P))
    W2 = W0 + W1
    C1 = NB * W2

    idx_arr = np.zeros((NCORES, 16, NB * 8 * W2), np.int16)
    ldst_arr = np.full((NCORES, P, C1), 255, dtype=np.uint8)
    for c in range(NCORES):
        for b in range(NB):
            p0s, p0l, p1s, p1l = parts[c * NB + b]
            i0 = np.zeros(W0 * P, np.int16)
            i0[:len(p0s)] = p0s
            i1 = np.zeros(W1 * P, np.int16)
            i1[:len(p1s)] = p1s
            col = b * 8 * W2
            idx_arr[c, :, col:col + 8 * W0] = _wrap16(i0)
            idx_arr[c, :, col + 8 * W0:col + 8 * W2] = _wrap16(i1)
            l0 = np.full(W0 * P, 255, np.uint8)
            l0[:len(p0l)] = p0l
            l1 = np.full(W1 * P, 255, np.uint8)
            l1[:len(p1l)] = p1l
            ldst_arr[c, :, b * W2:b * W2 + W0] = l0.reshape(W0, P).T
            ldst_arr[c, :, b * W2 + W0:(b + 1) * W2] = l1.reshape(W1, P).T

    x_pad = np.zeros((NP, IN_C), np.float16)
    x_pad[:N] = x.astype(np.float16)
    W1lT = np.ascontiguousarray(W1l.T.astype(np.float16))   # [128, 256]
    W1rT = np.ascontiguousarray(W1r.T.astype(np.float16))
    Wzo = np.zeros((P, 8), np.float16)
    for j in range(2):
        Wzo[:, 4 * j:4 * j + 2] = W2l.T[j * P:(j + 1) * P, :].astype(np.float16)
        Wzo[:, 4 * j + 2:4 * j + 4] = W2r.T[j * P:(j + 1) * P, :].astype(np.float16)
    b1p = np.ascontiguousarray(np.asarray(b1).reshape(2, P).T.astype(np.float32))
    b2b = np.tile(np.asarray(b2).reshape(1, 2), (P, 1)).astype(np.float32)
    recip_c = recip.reshape(NCORES, P, NB).copy()  # node local r = p*49+b

    in_maps = []
    for c in range(NCORES):
        in_maps.append({
            "x_sh": np.ascontiguousarray(x_pad[c * ROWS:(c + 1) * ROWS, :]),
            "idx16": np.ascontiguousarray(idx_arr[c]),
            "ldstu": np.ascontiguousarray(ldst_arr[c]),
            "W1lT": W1lT,
            "W1rT": W1rT,
            "Wzo": Wzo,
            "b1p": b1p,
            "b2b": b2b,
            "recip": np.ascontiguousarray(recip_c[c]),
        })
    return in_maps, W0, W1


def _build(W0, W1):
    W2 = W0 + W1
    C1 = NB * W2
    nc = bacc.Bacc(None, target_bir_lowering=False, debug=False)

    x_sh_d = nc.dram_tensor("x_sh", [ROWS, IN_C], f16, kind="ExternalInput")
    idx_d = nc.dram_tensor("idx16", [16, NB * 8 * W2], i16, kind="ExternalInput")
    ldstu_d = nc.dram_tensor("ldstu", [P, C1], u8, kind="ExternalInput")
    W1lT_d = nc.dram_tensor("W1lT", [P, HID], f16, kind="ExternalInput")
    W1rT_d = nc.dram_tensor("W1rT", [P, HID], f16, kind="ExternalInput")
    Wzo_d = nc.dram_tensor("Wzo", [P, 8], f16, kind="ExternalInput")
    b1p_d = nc.dram_tensor("b1p", [P, 2], f32, kind="ExternalInput")
    b2b_d = nc.dram_tensor("b2b", [P, 2], f32, kind="ExternalInput")
    recip_d = nc.dram_tensor("recip", [P, NB], f32, kind="ExternalInput")
    out_d = nc.dram_tensor("out", [P, 2 * NB], f32, kind="ExternalOutput")

    with tile.TileContext(nc) as tc:
        with (
            tc.tile_pool(name="big", bufs=1) as big,
            tc.tile_pool(name="lp", bufs=4) as lp,
            tc.tile_pool(name="pp", bufs=2, space="PSUM") as pp,
            tc.tile_pool(name="dram", bufs=1, space="DRAM") as dp,
        ):
            def load(d, shape, dt, tag):
                t = big.tile(shape, dt, tag=tag, name=tag)
                nc.sync.dma_start(out=t[:], in_=d[:, :])
                return t

            W1lT_sb = load(W1lT_d, [P, HID], f16, "w1l")
            W1rT_sb = load(W1rT_d, [P, HID], f16, "w1r")
            Wzo_sb = load(Wzo_d, [P, 8], f16, "wzo")
            b1_sb = load(b1p_d, [P, 2], f32, "b1")
            b2_sb = load(b2b_d, [P, 2], f32, "b2")
            recip_sb = load(recip_d, [P, NB], f32, "recip")

            # replicate the 16-partition index block across all 8 core groups
            idx_sb = big.tile([P, NB * 8 * W2], i16, tag="idx", name="idx_sb")
            for g in range(8):
                nc.sync.dma_start(
                    out=idx_sb[16 * g:16 * (g + 1), :], in_=idx_d[:, :]
                )

            # iota / identity built on device
            ioti = big.tile([P, P], i32, tag="ioti", name="ioti")
            nc.gpsimd.iota(out=ioti[:], pattern=[[1, P]], base=0,
                           channel_multiplier=0)
            iotp = big.tile([P, P], i32, tag="iotp", name="iotp")
            nc.gpsimd.iota(out=iotp[:], pattern=[[0, P]], base=0,
                           channel_multiplier=1)
            iota_sb = big.tile([P, P], f32, tag="iota", name="iota_sb")
            nc.vector.tensor_copy(out=iota_sb[:], in_=ioti[:])
            identh = big.tile([P, P], f16, tag="identh", name="identh")
            nc.vector.tensor_tensor(
                out=identh[:], in0=ioti[:], in1=iotp[:],
                op=mybir.AluOpType.is_equal,
            )

            # x shard -> internal DRAM -> AllGather to full x
            x_int = dp.tile([ROWS, IN_C], f16, tag="xint", name="x_int")
            nc.sync.dma_start(out=x_int[:, :], in_=x_sh_d[:, :])
            x_full = dp.tile([NP, IN_C], f16, tag="xfull", name="x_full",
                             addr_space="Shared")
            nc.gpsimd.collective_compute(
                "AllGather",
                mybir.AluOpType.bypass,
                replica_groups=[list(range(NCORES))],
                ins=[x_int[:, :]],
                outs=[x_full[:, :]],
            )
            xb_src = x_int[:, :].rearrange("(p b) c -> p b c", b=NB)

            # layer-2 z table: 256B-padded rows, node order
            z_own = dp.tile([ROWS, P], f16, tag="zown", name="z_own")
            z_own_v = z_own[:, :].rearrange("(p b) f -> p b f", b=NB)
            z_full = dp.tile([NP, P], f16, tag="zfull", name="z_full",
                             addr_space="Shared")
            o_stage = dp.tile([P, 2 * NB], f32, tag="ostage", name="o_stage")

            out_sb = big.tile([P, 2 * NB], f32, tag="outs", name="out_sb")

            with tc.For_i(0, NB, name="l1") as b:
                g0 = lp.tile([P, W0, IN_C], f16, tag="g0", name="g0")
                nc.gpsimd.dma_gather(
                    out_ap=g0[:, :, :],
                    in_ap=x_full[0:HALF, :],
                    idxs_ap=idx_sb[:, ds(b * 8 * W2, 8 * W0)],
                    num_idxs=W0 * P,
                    num_idxs_reg=W0 * P,
                    elem_size=IN_C,
                    single_packet=False,
                )
                g1 = lp.tile([P, W1, IN_C], f16, tag="g1", name="g1")
                nc.gpsimd.dma_gather(
                    out_ap=g1[:, :, :],
                    in_ap=x_full[HALF:NP, :],
                    idxs_ap=idx_sb[:, ds(b * 8 * W2 + 8 * W0, 8 * W1)],
                    num_idxs=W1 * P,
                    num_idxs_reg=W1 * P,
                    elem_size=IN_C,
                    single_packet=False,
                )
                ldb_u = lp.tile([P, W2], u8, tag="ldbu", name="ldb_u")
                nc.sync.dma_start(out=ldb_u[:], in_=ldstu_d[:, ds(b * W2, W2)])
                ldb = lp.tile([P, W2], f32, tag="ldb", name="ldb")
                nc.vector.tensor_copy(out=ldb[:], in_=ldb_u[:])

                pagg = pp.tile([P, P], f32, tag="agg", name="pagg")
                for k in range(W2):
                    Pt = lp.tile([P, P], f16, tag="P", name="Pt")
                    nc.vector.tensor_scalar(
                        out=Pt[:], in0=iota_sb[:],
                        scalar1=ldb[:, k:k + 1], scalar2=None,
                        op0=mybir.AluOpType.is_equal,
                    )
                    rhs = g0[:, k, :] if k < W0 else g1[:, k - W0, :]
                    nc.tensor.matmul(
                        out=pagg[:], lhsT=Pt[:], rhs=rhs,
                        start=(k == 0), stop=(k == W2 - 1),
                    )
                rcb = lp.tile([P, 1], f32, tag="rcb", name="rcb")
                nc.sync.dma_start(out=rcb[:], in_=recip_d[:, ds(b, 1)])
                aggm = lp.tile([P, P], f16, tag="aggm", name="aggm")
                nc.vector.tensor_scalar(
                    out=aggm[:], in0=pagg[:], scalar1=rcb[:, 0:1],
                    scalar2=None, op0=mybir.AluOpType.mult,
                )
                ptr = pp.tile([P, P], f16, tag="tr", name="ptr", bufs=3)
                nc.tensor.transpose(out=ptr[:], in_=aggm[:], identity=identh[:])
                aggmT = lp.tile([P, P], f16, tag="aggmT", name="aggmT")
                nc.vector.tensor_copy(out=aggmT[:], in_=ptr[:])

                xb = lp.tile([P, IN_C], f16, tag="xb", name="xb")
                nc.sync.dma_start(out=xb[:], in_=xb_src[:, ds(b, 1), :])
                ptr2 = pp.tile([P, P], f16, tag="tr", name="ptr2", bufs=3)
                nc.tensor.transpose(out=ptr2[:], in_=xb[:], identity=identh[:])
                xbT = lp.tile([P, P], f16, tag="xbT", name="xbT")
                nc.vector.tensor_copy(out=xbT[:], in_=ptr2[:])

                hbT = []
                for j in range(2):
                    ph = pp.tile([P, P], f32, tag="tr", name="ph", bufs=3)
                    nc.tensor.matmul(
                        out=ph[:], lhsT=W1lT_sb[:, j * P:(j + 1) * P],
                        rhs=aggmT[:], start=True, stop=False,
                    )
                    nc.tensor.matmul(
                        out=ph[:], lhsT=W1rT_sb[:, j * P:(j + 1) * P],
                        rhs=xbT[:], start=False, stop=True,
                    )
                    ht = lp.tile([P, P], f16, tag=f"hbT{j}", name=f"ht{j}")
                    nc.scalar.activation(
                        out=ht[:], in_=ph[:],
                        func=mybir.ActivationFunctionType.Relu,
                        bias=b1_sb[:, j:j + 1],
                    )
                    hbT.append(ht)
                pzo = pp.tile([P, 4], f32, tag="zo", name="pzo", bufs=1)
                for j in range(2):
                    nc.tensor.matmul(
                        out=pzo[:], lhsT=hbT[j][:],
                        rhs=Wzo_sb[:, 4 * j:4 * j + 4],
                        start=(j == 0), stop=(j == 1),
                    )
                zb = lp.tile([P, 2], f16, tag="zb", name="zb")
                nc.vector.tensor_copy(out=zb[:], in_=pzo[:, 0:2])
                nc.sync.dma_start(out=z_own_v[:, ds(b, 1), 0:2], in_=zb[:])
                ob = lp.tile([P, 2], f32, tag="ob", name="ob")
                nc.vector.tensor_tensor(
                    out=ob[:], in0=pzo[:, 2:4], in1=b2_sb[:],
                    op=mybir.AluOpType.add,
                )
                nc.sync.dma_start(out=o_stage[:, ts(b, 2)], in_=ob[:])

            nc.gpsimd.collective_compute(
                "AllGather",
                mybir.AluOpType.bypass,
                replica_groups=[list(range(NCORES))],
                ins=[z_own[:, :]],
                outs=[z_full[:, :]],
            )

            with tc.For_i(0, NB, name="l2") as b:
                zg0 = lp.tile([P, W0, P], f16, tag="zg0", name="zg0")
                nc.gpsimd.dma_gather(
                    out_ap=zg0[:, :, :],
                    in_ap=z_full[0:HALF, :],
                    idxs_ap=idx_sb[:, ds(b * 8 * W2, 8 * W0)],
                    num_idxs=W0 * P,
                    num_idxs_reg=W0 * P,
                    elem_size=P,
                    single_packet=False,
                )
                zg1 = lp.tile([P, W1, P], f16, tag="zg1", name="zg1")
                nc.gpsimd.dma_gather(
                    out_ap=zg1[:, :, :],
                    in_ap=z_full[HALF:NP, :],
                    idxs_ap=idx_sb[:, ds(b * 8 * W2 + 8 * W0, 8 * W1)],
                    num_idxs=W1 * P,
                    num_idxs_reg=W1 * P,
                    elem_size=P,
                    single_packet=False,
                )
                ldb_u = lp.tile([P, W2], u8, tag="ldbu", name="ldb_u2")
                nc.sync.dma_start(out=ldb_u[:], in_=ldstu_d[:, ds(b * W2, W2)])
                ldb = lp.tile([P, W2], f32, tag="ldb", name="ldb2")
                nc.vector.tensor_copy(out=ldb[:], in_=ldb_u[:])

                pa2 = pp.tile([P, 2], f32, tag="agg2", name="pa2")
                for k in range(W2):
                    P2 = lp.tile([P, P], f16, tag="P", name="P2")
                    nc.vector.tensor_scalar(
                        out=P2[:], in0=iota_sb[:],
                        scalar1=ldb[:, k:k + 1], scalar2=None,
                        op0=mybir.AluOpType.is_equal,
                    )
                    rhs = (zg0[:, k, 0:2] if k < W0 else zg1[:, k - W0, 0:2])
                    nc.tensor.matmul(
                        out=pa2[:], lhsT=P2[:], rhs=rhs,
                        start=(k == 0), stop=(k == W2 - 1),
                    )
                rcb = lp.tile([P, 1], f32, tag="rcb", name="rcb2")
                nc.sync.dma_start(out=rcb[:], in_=recip_d[:, ds(b, 1)])
                red2 = lp.tile([P, 2], f32, tag="red2", name="red2")
                nc.vector.tensor_scalar(
                    out=red2[:], in0=pa2[:], scalar1=rcb[:, 0:1],
                    scalar2=None, op0=mybir.AluOpType.mult,
                )
                ob = lp.tile([P, 2], f32, tag="ob", name="ob2")
                nc.sync.dma_start(out=ob[:], in_=o_stage[:, ts(b, 2)])
                outb = lp.tile([P, 2], f32, tag="outb", name="outb")
                nc.vector.tensor_tensor(
                    out=outb[:], in0=red2[:], in1=ob[:],
                    op=mybir.AluOpType.add,
                )
                nc.vector.tensor_copy(out=out_sb[:, ts(b, 2)], in_=outb[:])

            nc.sync.dma_start(out=out_d[:, :], in_=out_sb[:])
    nc.compile()
    return nc


def _run(inputs, repeat=1):
    in_maps, W0, W1 = _host_prep(**inputs)
    nc = _build(W0, W1)
    best = None
    for _ in range(repeat):
        t0 = time.perf_counter()
        res = run_bass_kernel_spmd(
            nc, [dict(m) for m in in_maps], core_ids=list(range(NCORES))
        )
        dt = time.perf_counter() - t0
        print(f"  spmd run: {dt:.3f}s", flush=True)
        best = dt if best is None else min(best, dt)
    outs = []
    for c in range(NCORES):
        a = res.results[c]["out"]  # [128, 98]; row p, col 2b+f = node p*49+b
        outs.append(a.reshape(ROWS, 2))
    full = np.concatenate(outs, axis=0)[:N]
    return full.astype(np.float32), best


def kernel(**inputs):
    out, _ = _run(inputs, repeat=1)
    return out
